# revision 3
# baseline (speedup 1.0000x reference)
"""Additive (Bahdanau) attention on 8 TRN2 NeuronCores, data-parallel over batch.

Reference computation (per batch b):
    q_proj = query @ W1_w.T + W1_b                      # [dim]
    k_proj = key @ W2_w.T + W2_b                        # [S, dim]
    h      = tanh(k_proj + q_proj)                      # [S, dim]
    score  = h @ V_w[0] + V_b                           # [S]   (V_b cancels in softmax)
    attn   = softmax(score)                             # [S]
    context= attn @ value                               # [dim]

Device mapping (per core, B_LOC=4 batches):
  - keyT [4, D, S] bf16 (host-transposed) so the big matmul contracts d on
    partitions: k_projT tile [128 e, 512 s] = sum_d W2T[d,e] . keyT[d,s].
  - tanh fused on ScalarE with per-partition bias qb[e,b] = q_proj[e,b]+W1_b+W2_b.
  - score via M=1 matmuls (lhsT=V column); per-strip exp with partial sums;
    unnormalized exp row -> per-s-tile columns via K=1 broadcast matmuls;
    context accumulated over s-tiles, normalized by 1/sumexp in the epilogue.
  - 16 s-strips (4 batches x 4 strips) in one software pipeline; the broadcast/
    context matmuls of strip i are emitted at strip i+2 so their ACT/DVE
    producers are long done when the PE reaches them.
  - DMA queues: sync=keyT strips, scalar=w2t+outputs, gpsimd=w1t/query/biases/value.
"""

import os
import numpy as np
import ml_dtypes
from contextlib import ExitStack

import concourse.bass as bass
import concourse.mybir as mybir
import concourse.tile as tile
from concourse import bacc
from concourse.bass import ts
from concourse.bass_utils import run_bass_kernel_spmd

BF16 = mybir.dt.bfloat16
F32 = mybir.dt.float32
AF = mybir.ActivationFunctionType

N_CORES = 8
B, S, D = 32, 2048, 1024          # full problem
B_LOC = B // N_CORES              # 4 batches per core
P = 128                           # partitions
NCH = D // P                      # 8 chunks of 128 along d and e
SW = 512                          # s-strip width (one PSUM bank at f32)
NSTRIP = S // SW                  # 4 strips per batch
STILE = S // P                    # 16 s-tiles of 128

_CACHE = {}
LAST_EXEC_NS = None


def _build_nc():
    nc = bacc.Bacc("TRN2", target_bir_lowering=False, debug=False,
                   num_devices=N_CORES)
    keyT = nc.declare_dram_parameter("keyT", [B_LOC, D, S], BF16, isOutput=False)
    value = nc.declare_dram_parameter("value", [B_LOC, S, D], BF16, isOutput=False)
    w2t = nc.declare_dram_parameter("w2t", [D, D], BF16, isOutput=False)
    w1t = nc.declare_dram_parameter("w1t", [D, D], BF16, isOutput=False)
    queryT = nc.declare_dram_parameter("queryT", [D, B_LOC], BF16, isOutput=False)
    w1b = nc.declare_dram_parameter("w1b", [D], F32, isOutput=False)
    w2b = nc.declare_dram_parameter("w2b", [D], F32, isOutput=False)
    vw = nc.declare_dram_parameter("vw", [D], BF16, isOutput=False)
    ctx_o = nc.declare_dram_parameter("ctxo", [B_LOC, D], F32, isOutput=True)
    attn_o = nc.declare_dram_parameter("attn", [B_LOC, S], F32, isOutput=True)

    with ExitStack() as ctx:
        tc = ctx.enter_context(tile.TileContext(nc))
        const = ctx.enter_context(tc.tile_pool(name="const", bufs=1))
        kpool = ctx.enter_context(tc.tile_pool(name="kpool", bufs=2))
        hpool = ctx.enter_context(tc.tile_pool(name="hpool", bufs=2))
        vpool = ctx.enter_context(tc.tile_pool(name="vpool", bufs=3))
        rows = ctx.enter_context(tc.tile_pool(name="rows", bufs=2))
        stat = ctx.enter_context(tc.tile_pool(name="stat", bufs=2))
        ecolp = ctx.enter_context(tc.tile_pool(name="ecolp", bufs=8))
        ctxop = ctx.enter_context(tc.tile_pool(name="ctxop", bufs=2))
        psA = ctx.enter_context(tc.tile_pool(name="psA", bufs=2, space="PSUM"))
        psS = ctx.enter_context(tc.tile_pool(name="psS", bufs=2, space="PSUM"))
        psB = ctx.enter_context(tc.tile_pool(name="psB", bufs=2, space="PSUM"))
        psC = ctx.enter_context(tc.tile_pool(name="psC", bufs=1, space="PSUM"))

        # ---- HAM warmup: dummy matmuls on a zeroed tile while the first
        # keyT strip is still in flight, so real matmuls start at 2.4 GHz.
        warm_sb = const.tile([P, SW], BF16)
        nc.vector.memset(warm_sb, 0.0)
        wps = psA.tile([P, SW], F32, tag="pk", name="warm")
        for w in range(26):
            nc.tensor.matmul(wps, lhsT=warm_sb[:, 0:P], rhs=warm_sb,
                             start=(w == 0), stop=(w == 25))

        # ---- resident weights / constants (scalar + gpsimd DMA queues; the
        # sync queue is reserved for the keyT strips on the critical path).
        w2t_sb = const.tile([P, NCH, D], BF16)
        nc.scalar.dma_start(out=w2t_sb, in_=w2t.ap().rearrange("(do di) e -> di do e", di=P))
        w1t_sb = const.tile([P, NCH, D], BF16)
        nc.gpsimd.dma_start(out=w1t_sb, in_=w1t.ap().rearrange("(do di) e -> di do e", di=P))
        q_sb = const.tile([P, NCH, B_LOC], BF16)
        nc.gpsimd.dma_start(out=q_sb, in_=queryT.ap().rearrange("(do di) b -> di do b", di=P))
        w1b_sb = const.tile([P, NCH], F32)
        nc.gpsimd.dma_start(out=w1b_sb, in_=w1b.ap().rearrange("(do di) -> di do", di=P))
        w2b_sb = const.tile([P, NCH], F32)
        nc.gpsimd.dma_start(out=w2b_sb, in_=w2b.ap().rearrange("(do di) -> di do", di=P))
        vw_sb = const.tile([P, NCH], BF16)
        nc.gpsimd.dma_start(out=vw_sb, in_=vw.ap().rearrange("(do di) -> di do", di=P))
        ones_sb = const.tile([1, 1], BF16)
        nc.vector.memset(ones_sb, 1.0)
        biassum = const.tile([P, NCH], F32)
        nc.vector.tensor_add(biassum, w1b_sb, w2b_sb)
        qb_sb = const.tile([P, NCH, B_LOC], F32)

        def emit_qproj():
            for j in range(NCH):
                pq = psB.tile([P, B_LOC], F32, tag="small", name=f"pq_{j}")
                for i in range(NCH):
                    nc.tensor.matmul(pq, lhsT=w1t_sb[:, i, ts(j, P)], rhs=q_sb[:, i, :],
                                     start=(i == 0), stop=(i == NCH - 1))
                nc.vector.tensor_scalar_add(qb_sb[:, j, :], pq, biassum[:, j:j + 1])

        # per-batch state
        score_sb = [None] * B_LOC
        exp_sb = [None] * B_LOC
        partials = [None] * B_LOC
        pcs = [None] * B_LOC
        vts = [[None] * NSTRIP for _ in range(B_LOC)]
        ecs = {}

        def emit_bcast(b, t):
            # broadcast exp row slices into per-s-tile columns (K=1 matmuls)
            for kk in range(NSTRIP):
                k = NSTRIP * t + kk
                pb = psB.tile([P, 1], F32, tag="small", name=f"pb_{b}_{k}")
                nc.tensor.matmul(pb, lhsT=exp_sb[b][0:1, ts(k, P)], rhs=ones_sb,
                                 start=True, stop=True)
                ec = ecolp.tile([P, 1], BF16, tag="ecol", name=f"ec_{b}_{k}")
                nc.vector.tensor_copy(ec, pb)
                ecs[(b, k)] = ec

        def emit_ctx(b, t):
            for kk in range(NSTRIP):
                k = NSTRIP * t + kk
                ec = ecs.pop((b, k))
                for n in range(2):
                    nc.tensor.matmul(pcs[b][:, ts(n, SW)], lhsT=ec,
                                     rhs=vts[b][t][:, kk, ts(n, SW)],
                                     start=(k == 0), stop=(k == STILE - 1))

        def emit_finalize(b):
            ssum = stat.tile([1, 1], F32, tag="ssum", name=f"ssum_{b}")
            nc.vector.reduce_sum(out=ssum, in_=partials[b], axis=mybir.AxisListType.X)
            recip = stat.tile([1, 1], F32, tag="recip", name=f"recip_{b}")
            nc.vector.reciprocal(recip, ssum)
            attn_sb = rows.tile([1, S], F32, tag="attn", name=f"attn_{b}")
            nc.vector.tensor_scalar_mul(attn_sb, exp_sb[b], recip)
            nc.scalar.dma_start(out=attn_o.ap()[b:b + 1, :], in_=attn_sb)
            ctx_sb = ctxop.tile([1, D], F32, tag="ctxo", name=f"ctxo_{b}")
            for n in range(2):
                nc.scalar.activation(out=ctx_sb[:, ts(n, SW)], in_=pcs[b][:, ts(n, SW)],
                                     func=AF.Copy, bias=0.0, scale=recip[:, 0:1])
            nc.scalar.dma_start(out=ctx_o.ap()[b:b + 1, :], in_=ctx_sb)

        def emit_strip(b, t):
            if t == 0:
                score_sb[b] = rows.tile([1, S], F32, tag="score", name=f"score_{b}")
                exp_sb[b] = rows.tile([1, S], BF16, tag="exp", name=f"exp_{b}")
                partials[b] = stat.tile([1, NSTRIP], F32, tag="part", name=f"part_{b}")
                pcs[b] = psC.tile([1, 2 * SW], F32, tag="pc", name=f"pc_{b}")
            kt = kpool.tile([P, NCH, SW], BF16, tag="kT", name=f"kt_{b}_{t}")
            nc.sync.dma_start(
                out=kt,
                in_=keyT.ap()[b].rearrange("(do di) s -> di do s", di=P)[:, :, ts(t, SW)])
            vt = vpool.tile([P, NSTRIP, D], BF16, tag="vt", name=f"vt_{b}_{t}")
            nc.gpsimd.dma_start(
                out=vt,
                in_=value.ap()[b].rearrange("(ko ki) d -> ki ko d", ki=P)[:, ts(t, NSTRIP), :])
            vts[b][t] = vt
            h = hpool.tile([P, NCH, SW], BF16, tag="h", name=f"h_{b}_{t}")
            for j in range(NCH):
                pk = psA.tile([P, SW], F32, tag="pk", name=f"pk_{b}_{t}_{j}")
                for i in range(NCH):
                    nc.tensor.matmul(pk, lhsT=w2t_sb[:, i, ts(j, P)], rhs=kt[:, i, :],
                                     start=(i == 0), stop=(i == NCH - 1))
                if b == 0 and t == 0 and j == 0:
                    emit_qproj()
                nc.scalar.activation(out=h[:, j, :], in_=pk, func=AF.Tanh,
                                     bias=qb_sb[:, j, b:b + 1], scale=1.0)
            ps_s = psS.tile([1, SW], F32, tag="pss", name=f"pss_{b}_{t}")
            for j in range(NCH):
                nc.tensor.matmul(ps_s, lhsT=vw_sb[:, j:j + 1], rhs=h[:, j, :],
                                 start=(j == 0), stop=(j == NCH - 1))
            nc.vector.tensor_copy(score_sb[b][:, ts(t, SW)], ps_s)
            nc.scalar.activation(out=exp_sb[b][:, ts(t, SW)], in_=score_sb[b][:, ts(t, SW)],
                                 func=AF.Exp, bias=0.0, scale=1.0,
                                 accum_out=partials[b][:, t:t + 1])

        strips = [(b, t) for b in range(B_LOC) for t in range(NSTRIP)]
        for idx, (b, t) in enumerate(strips):
            if idx >= 2:
                db, dt_ = strips[idx - 2]
                emit_bcast(db, dt_)
                emit_ctx(db, dt_)
                if dt_ == NSTRIP - 1:
                    emit_finalize(db)
            emit_strip(b, t)
        for idx in (len(strips) - 2, len(strips) - 1):
            db, dt_ = strips[idx]
            emit_bcast(db, dt_)
            emit_ctx(db, dt_)
            if dt_ == NSTRIP - 1:
                emit_finalize(db)

    nc.compile()
    return nc


def kernel(query, key, value, W1_w, W1_b, W2_w, W2_b, V_w, V_b):
    global LAST_EXEC_NS
    query = np.asarray(query, dtype=np.float32)
    key = np.asarray(key, dtype=np.float32)
    value = np.asarray(value, dtype=np.float32)
    W1_w = np.asarray(W1_w, dtype=np.float32)
    W1_b = np.asarray(W1_b, dtype=np.float32)
    W2_w = np.asarray(W2_w, dtype=np.float32)
    W2_b = np.asarray(W2_b, dtype=np.float32)
    V_w = np.asarray(V_w, dtype=np.float32)

    if "nc" not in _CACHE:
        _CACHE["nc"] = _build_nc()
    nc = _CACHE["nc"]

    bf = ml_dtypes.bfloat16
    w2t = np.ascontiguousarray(W2_w.T).astype(bf)
    w1t = np.ascontiguousarray(W1_w.T).astype(bf)
    vw_ = V_w[0].astype(bf)
    key_bf = key.astype(bf)
    val_bf = value.astype(bf)
    qT = np.ascontiguousarray(query.T).astype(bf)

    in_maps = []
    for c in range(N_CORES):
        sl = slice(B_LOC * c, B_LOC * (c + 1))
        in_maps.append({
            "keyT": np.ascontiguousarray(key_bf[sl].transpose(0, 2, 1)),
            "value": val_bf[sl],
            "w2t": w2t,
            "w1t": w1t,
            "queryT": np.ascontiguousarray(qT[:, sl]),
            "w1b": W1_b,
            "w2b": W2_b,
            "vw": vw_,
        })

    res = run_bass_kernel_spmd(nc, in_maps, core_ids=list(range(N_CORES)))
    LAST_EXEC_NS = res.exec_time_ns

    context = np.concatenate([res.results[c]["ctxo"] for c in range(N_CORES)], axis=0)
    attn = np.concatenate([res.results[c]["attn"] for c in range(N_CORES)], axis=0)
    return (context.astype(np.float32), attn.astype(np.float32))


# revision 23
# speedup vs baseline: 1.2855x; 1.2855x over previous
"""Additive (Bahdanau) attention on 8 TRN2 NeuronCores, data-parallel over batch.

Reference computation (per batch b):
    q_proj = query @ W1_w.T + W1_b                      # [dim]
    k_proj = key @ W2_w.T + W2_b                        # [S, dim]
    h      = tanh(k_proj + q_proj)                      # [S, dim]
    score  = h @ V_w[0] + V_b                           # [S]   (V_b cancels in softmax)
    attn   = softmax(score)                             # [S]
    context= attn @ value                               # [dim]

Device mapping (per core, B_LOC=4 batches):
  - keyT [4, D, S] bf16 (host-transposed) so the big matmul contracts d on
    partitions: k_projT tile [128 e, 512 s] = sum_d W2T[d,e] . keyT[d,s].
  - tanh fused on ScalarE with per-partition bias qb[e,b] = q_proj[e,b]+W1_b+W2_b.
  - score via M=1 matmuls (lhsT=V column); per-strip exp with partial sums;
    unnormalized exp row -> per-s-tile columns via K=1 broadcast matmuls;
    context accumulated over s-tiles, normalized by 1/sumexp in the epilogue.
  - 16 s-strips (4 batches x 4 strips) in one software pipeline; the broadcast/
    context matmuls of strip i are emitted at strip i+2 so their ACT/DVE
    producers are long done when the PE reaches them.
  - single priority-ordered DMA FIFO (scalar HWDGE): small bundles, w2t,
    kt(0,0), w1t, then per strip kt(i) followed by value(i-1); all inputs are
    host-pre-swizzled so each DMA is a fat contiguous per-partition read.
  - HAM warmup: dummy matmuls on a zeroed tile cover the initial DMA window so
    real matmuls start at full clock.
"""

import numpy as np
import ml_dtypes
from contextlib import ExitStack

import concourse.mybir as mybir
import concourse.tile as tile
from concourse import bacc
from concourse.bass import ts
from concourse.bass_utils import run_bass_kernel_spmd

BF16 = mybir.dt.bfloat16
F32 = mybir.dt.float32
AF = mybir.ActivationFunctionType

N_CORES = 8
B, S, D = 32, 2048, 1024          # full problem
B_LOC = B // N_CORES              # 4 batches per core
P = 128                           # partitions
NCH = D // P                      # 8 chunks of 128 along d and e
SW = 512                          # s-strip width (one PSUM bank at f32)
NSTRIP = S // SW                  # 4 strips per batch
STILE = S // P                    # 16 s-tiles of 128

_CACHE = {}
LAST_EXEC_NS = None


def _build_nc():
    nc = bacc.Bacc("TRN2", target_bir_lowering=False, debug=False,
                   num_devices=N_CORES)
    # all big inputs are host-pre-swizzled into the exact SBUF tile layout so
    # every DMA is a fat contiguous per-partition read (128 x 8-16KB descriptors)
    keyT = nc.declare_dram_parameter("keyT", [B_LOC, NSTRIP, P, NCH, SW], BF16, isOutput=False)
    value = nc.declare_dram_parameter("value", [B_LOC, NSTRIP, P, NSTRIP, D], BF16, isOutput=False)
    w2t = nc.declare_dram_parameter("w2t", [P, NCH, D], BF16, isOutput=False)
    w1t = nc.declare_dram_parameter("w1t", [P, NCH, D], BF16, isOutput=False)
    # pre-swizzled host bundles: one contiguous row per partition
    sm_f32 = nc.declare_dram_parameter("sm_f32", [P, 2 * NCH], F32, isOutput=False)
    sm_bf16 = nc.declare_dram_parameter("sm_bf16", [P, NCH + NCH * B_LOC], BF16, isOutput=False)
    ctx_o = nc.declare_dram_parameter("ctxo", [B_LOC, D], F32, isOutput=True)
    attn_o = nc.declare_dram_parameter("attn", [B_LOC, S], F32, isOutput=True)

    with ExitStack() as ctx:
        tc = ctx.enter_context(tile.TileContext(nc))
        const = ctx.enter_context(tc.tile_pool(name="const", bufs=1))
        kpool = ctx.enter_context(tc.tile_pool(name="kpool", bufs=2))
        hpool = ctx.enter_context(tc.tile_pool(name="hpool", bufs=2))
        vpool = ctx.enter_context(tc.tile_pool(name="vpool", bufs=3))
        rows = ctx.enter_context(tc.tile_pool(name="rows", bufs=2))
        stat = ctx.enter_context(tc.tile_pool(name="stat", bufs=2))
        ecolp = ctx.enter_context(tc.tile_pool(name="ecolp", bufs=8))
        ctxop = ctx.enter_context(tc.tile_pool(name="ctxop", bufs=2))
        psA = ctx.enter_context(tc.tile_pool(name="psA", bufs=3, space="PSUM"))
        psS = ctx.enter_context(tc.tile_pool(name="psS", bufs=1, space="PSUM"))
        psB = ctx.enter_context(tc.tile_pool(name="psB", bufs=2, space="PSUM"))
        psC = ctx.enter_context(tc.tile_pool(name="psC", bufs=1, space="PSUM"))

        # ---- HAM warmup: dummy matmuls on a zeroed tile while the first
        # keyT strip is still in flight, so real matmuls start at 2.4 GHz.
        warm_sb = const.tile([P, SW], BF16)
        nc.vector.memset(warm_sb, 0.0)
        wps = psA.tile([P, SW], F32, tag="pk", name="warm")
        for w in range(40):
            nc.tensor.matmul(wps, lhsT=warm_sb[:, 0:P], rhs=warm_sb,
                             start=(w == 0), stop=(w == 39))

        # ---- resident weights / constants on the single scalar DMA FIFO
        smf_sb = const.tile([P, 2 * NCH], F32)
        nc.scalar.dma_start(out=smf_sb, in_=sm_f32.ap())
        smb_sb = const.tile([P, NCH + NCH * B_LOC], BF16)
        nc.scalar.dma_start(out=smb_sb, in_=sm_bf16.ap())
        w2t_sb = const.tile([P, NCH, D], BF16)
        nc.scalar.dma_start(out=w2t_sb, in_=w2t.ap())
        # w1t's DMA is emitted inside the first strip, after that strip's keyT
        # load, so the FIFO delivers w2t+kt(0,0) (which gate the main matmuls)
        # first; qproj (the only w1t consumer) runs after the first 3 groups
        w1t_sb = const.tile([P, NCH, D], BF16)
        w1b_sb = smf_sb[:, 0:NCH]
        w2b_sb = smf_sb[:, NCH:2 * NCH]
        vw_sb = smb_sb[:, 0:NCH]
        q_sb = smb_sb[:, NCH:].rearrange("p (do b) -> p do b", b=B_LOC)
        ones_sb = const.tile([1, 1], BF16)
        nc.vector.memset(ones_sb, 1.0)
        biassum = const.tile([P, NCH], F32)
        nc.vector.tensor_add(biassum, w1b_sb, w2b_sb)
        qb_sb = const.tile([P, NCH, B_LOC], F32)

        def emit_qproj():
            for j in range(NCH):
                pq = psB.tile([P, B_LOC], F32, tag="small", name=f"pq_{j}")
                for i in range(NCH):
                    nc.tensor.matmul(pq, lhsT=w1t_sb[:, i, ts(j, P)], rhs=q_sb[:, i, :],
                                     start=(i == 0), stop=(i == NCH - 1))
                nc.vector.tensor_scalar_add(qb_sb[:, j, :], pq, biassum[:, j:j + 1])

        # per-batch state
        score_sb = [None] * B_LOC
        exp_sb = [None] * B_LOC
        partials = [None] * B_LOC
        pcs = [None] * B_LOC
        vts = [[None] * NSTRIP for _ in range(B_LOC)]
        ecs = {}

        def emit_bcast(b, t):
            # broadcast exp row slices into per-s-tile columns (K=1 matmuls)
            for kk in range(NSTRIP):
                k = NSTRIP * t + kk
                pb = psB.tile([P, 1], F32, tag="small", name=f"pb_{b}_{k}")
                nc.tensor.matmul(pb, lhsT=exp_sb[b][0:1, ts(k, P)], rhs=ones_sb,
                                 start=True, stop=True)
                ec = ecolp.tile([P, 1], BF16, tag="ecol", name=f"ec_{b}_{k}")
                nc.vector.tensor_copy(ec, pb)
                ecs[(b, k)] = ec

        def emit_ctx(b, t):
            for kk in range(NSTRIP):
                k = NSTRIP * t + kk
                ec = ecs.pop((b, k))
                for n in range(2):
                    nc.tensor.matmul(pcs[b][:, ts(n, SW)], lhsT=ec,
                                     rhs=vts[b][t][:, kk, ts(n, SW)],
                                     start=(k == 0), stop=(k == STILE - 1))

        def emit_finalize(b):
            ssum = stat.tile([1, 1], F32, tag="ssum", name=f"ssum_{b}")
            nc.vector.reduce_sum(out=ssum, in_=partials[b], axis=mybir.AxisListType.X)
            recip = stat.tile([1, 1], F32, tag="recip", name=f"recip_{b}")
            nc.vector.reciprocal(recip, ssum)
            attn_sb = rows.tile([1, S], F32, tag="attn", name=f"attn_{b}")
            nc.vector.tensor_scalar_mul(attn_sb, exp_sb[b], recip)
            nc.scalar.dma_start(out=attn_o.ap()[b:b + 1, :], in_=attn_sb)
            ctx_sb = ctxop.tile([1, D], F32, tag="ctxo", name=f"ctxo_{b}")
            for n in range(2):
                nc.scalar.activation(out=ctx_sb[:, ts(n, SW)], in_=pcs[b][:, ts(n, SW)],
                                     func=AF.Copy, bias=0.0, scale=recip[:, 0:1])
            nc.scalar.dma_start(out=ctx_o.ap()[b:b + 1, :], in_=ctx_sb)

        def emit_strip(b, t):
            if t == 0:
                score_sb[b] = rows.tile([1, S], F32, tag="score", name=f"score_{b}")
                exp_sb[b] = rows.tile([1, S], BF16, tag="exp", name=f"exp_{b}")
                partials[b] = stat.tile([1, NSTRIP], F32, tag="part", name=f"part_{b}")
                pcs[b] = psC.tile([1, 2 * SW], F32, tag="pc", name=f"pc_{b}")
            kt = kpool.tile([P, NCH, SW], BF16, tag="kT", name=f"kt_{b}_{t}")
            nc.scalar.dma_start(out=kt, in_=keyT.ap()[b, t])
            if b == 0 and t == 0:
                nc.scalar.dma_start(out=w1t_sb, in_=w1t.ap())
            h = hpool.tile([P, NCH, SW], BF16, tag="h", name=f"h_{b}_{t}")
            pks = []
            for j in range(NCH):
                pk = psA.tile([P, SW], F32, tag="pk", name=f"pk_{b}_{t}_{j}")
                pks.append(pk)
                for i in range(NCH):
                    nc.tensor.matmul(pk, lhsT=w2t_sb[:, i, ts(j, P)], rhs=kt[:, i, :],
                                     start=(i == 0), stop=(i == NCH - 1))
                if b == 0 and t == 0 and j == 2:
                    emit_qproj()   # tanh j=0..2 emitted after (RAW on qb_sb)
                    for jj in range(3):
                        nc.scalar.activation(out=h[:, jj, :], in_=pks[jj], func=AF.Tanh,
                                             bias=qb_sb[:, jj, b:b + 1], scale=1.0)
                elif not (b == 0 and t == 0 and j < 2):
                    nc.scalar.activation(out=h[:, j, :], in_=pk, func=AF.Tanh,
                                         bias=qb_sb[:, j, b:b + 1], scale=1.0)
            ps_s = psS.tile([1, SW], F32, tag="pss", name=f"pss_{b}_{t}")
            for j in range(NCH):
                nc.tensor.matmul(ps_s, lhsT=vw_sb[:, j:j + 1], rhs=h[:, j, :],
                                 start=(j == 0), stop=(j == NCH - 1))
            nc.vector.tensor_copy(score_sb[b][:, ts(t, SW)], ps_s)
            nc.scalar.activation(out=exp_sb[b][:, ts(t, SW)], in_=score_sb[b][:, ts(t, SW)],
                                 func=AF.Exp, bias=0.0, scale=1.0,
                                 accum_out=partials[b][:, t:t + 1])

        def emit_vt(b, t):
            # value chunk for strip (b, t); consumed by emit_ctx at strip idx+2.
            # Queued on the single DMA FIFO *behind* that iteration's keyT
            # strip: the FIFO order delays it past the head's saturated window.
            vt = vpool.tile([P, NSTRIP, D], BF16, tag="vt", name=f"vt_{b}_{t}")
            nc.scalar.dma_start(out=vt, in_=value.ap()[b, t])
            vts[b][t] = vt

        strips = [(b, t) for b in range(B_LOC) for t in range(NSTRIP)]
        for idx, (b, t) in enumerate(strips):
            if idx >= 2:
                db, dt_ = strips[idx - 2]
                emit_bcast(db, dt_)
                emit_ctx(db, dt_)
                if dt_ == NSTRIP - 1:
                    emit_finalize(db)
            emit_strip(b, t)
            if idx >= 1:
                emit_vt(*strips[idx - 1])
        emit_vt(*strips[-1])
        for idx in (len(strips) - 2, len(strips) - 1):
            db, dt_ = strips[idx]
            emit_bcast(db, dt_)
            emit_ctx(db, dt_)
            if dt_ == NSTRIP - 1:
                emit_finalize(db)

    nc.compile()
    return nc


def kernel(query, key, value, W1_w, W1_b, W2_w, W2_b, V_w, V_b):
    global LAST_EXEC_NS
    query = np.asarray(query, dtype=np.float32)
    key = np.asarray(key, dtype=np.float32)
    value = np.asarray(value, dtype=np.float32)
    W1_w = np.asarray(W1_w, dtype=np.float32)
    W1_b = np.asarray(W1_b, dtype=np.float32)
    W2_w = np.asarray(W2_w, dtype=np.float32)
    W2_b = np.asarray(W2_b, dtype=np.float32)
    V_w = np.asarray(V_w, dtype=np.float32)

    if "nc" not in _CACHE:
        _CACHE["nc"] = _build_nc()
    nc = _CACHE["nc"]

    bf = ml_dtypes.bfloat16
    w2t = np.ascontiguousarray(W2_w.T.reshape(NCH, P, D).transpose(1, 0, 2)).astype(bf)
    w1t = np.ascontiguousarray(W1_w.T.reshape(NCH, P, D).transpose(1, 0, 2)).astype(bf)
    key_bf = key.astype(bf)
    val_bf = value.astype(bf)
    sm_f32 = np.concatenate(
        [W1_b.reshape(NCH, P).T, W2_b.reshape(NCH, P).T], axis=1)
    sm_f32 = np.ascontiguousarray(sm_f32, dtype=np.float32)
    vw_cols = V_w[0].reshape(NCH, P).T.astype(bf)   # [128, 8]

    in_maps = []
    for c in range(N_CORES):
        sl = slice(B_LOC * c, B_LOC * (c + 1))
        qT = query[sl].T.astype(bf)                  # [1024, 4]
        q_sw = qT.reshape(NCH, P, B_LOC).transpose(1, 0, 2).reshape(P, NCH * B_LOC)
        kT = key_bf[sl].transpose(0, 2, 1).reshape(B_LOC, NCH, P, NSTRIP, SW)
        kT = np.ascontiguousarray(kT.transpose(0, 3, 2, 1, 4))
        vv = val_bf[sl].reshape(B_LOC, NSTRIP, NSTRIP, P, D)
        vv = np.ascontiguousarray(vv.transpose(0, 1, 3, 2, 4))
        in_maps.append({
            "keyT": kT,
            "value": vv,
            "w2t": w2t,
            "w1t": w1t,
            "sm_f32": sm_f32,
            "sm_bf16": np.ascontiguousarray(np.concatenate([vw_cols, q_sw], axis=1)),
        })

    res = run_bass_kernel_spmd(nc, in_maps, core_ids=list(range(N_CORES)))
    LAST_EXEC_NS = res.exec_time_ns

    context = np.concatenate([res.results[c]["ctxo"] for c in range(N_CORES)], axis=0)
    attn = np.concatenate([res.results[c]["attn"] for c in range(N_CORES)], axis=0)
    return (context.astype(np.float32), attn.astype(np.float32))

